# revision 8
# baseline (speedup 1.0000x reference)
"""BitNet attention (B=2, S=2048, H=1024, 16 heads x 64) on 8 trn2 NeuronCores.

Sharding: core c -> batch b=c//4, heads [4*(c%4), 4*(c%4)+4) (feature slice of
256 columns of each projection). Per core the device computes:
  qT/kT = (x @ sign(Wq/Wk)^T)^T slices   [F=256, S]   (feat on partitions)
  v     = x @ sign(Wv)^T slice           [S, F]       (seq on partitions)
  S^T[k,q] = k_h q_h^T   (per head, k on partitions -> transpose-free PV)
  P^T = exp(c_score * S^T)               (no max subtraction; scores are O(5))
  ctx_aug^T = [v | 1]^T P^T              (ones column => softmax denominators)
  attn^T = P^T * (1/denominator) broadcast over partitions (PE ones-outer-product)
  out_partial = ctx^T.T @ sign(Wo)^T slice * (alpha_v*alpha_o)
Host: transposes attn^T back, sums the 4 partial outs per batch.
"""

import os
import threading

import numpy as np

NUM_HEADS = 16
HEAD_DIM = 64
P = 128


def build_kernel(
    S,
    H,
    NH,  # heads per core
    D,  # head dim
    QC,  # q-chunk size (free dim of attention tiles)
    c_score,  # alpha_q * alpha_k / sqrt(D)
    c_out,  # alpha_v * alpha_o
    use_f32r=True,
):
    """Build the single-core Bass program (same NEFF for all cores)."""
    import concourse.mybir as mybir
    from concourse import bacc
    from concourse.tile import TileContext

    F = NH * D  # feature slice width
    nKH = H // P  # contraction tiles for projections
    nST = S // P  # seq tiles (also attention k tiles)
    nQC = S // QC  # q chunks
    nMF = F // P  # feature tiles
    VW = NH * (D + 1)  # v_aug width per seq tile
    JC = min(512, H)  # out-proj column chunk
    NJ = H // JC

    f32 = mybir.dt.float32
    di = mybir.dt.float32r if use_f32r else f32

    nc = bacc.Bacc(None, target_bir_lowering=False)
    xT = nc.dram_tensor("xT", [H, S], di, kind="ExternalInput")
    wqT = nc.dram_tensor("wqT", [H, F], di, kind="ExternalInput")
    wkT = nc.dram_tensor("wkT", [H, F], di, kind="ExternalInput")
    wvT = nc.dram_tensor("wvT", [H, F], di, kind="ExternalInput")
    woT = nc.dram_tensor("woT", [F, H], di, kind="ExternalInput")
    attn_t = nc.dram_tensor("attn_t", [NH, S, S], di, kind="ExternalOutput")
    out_p = nc.dram_tensor("out_p", [S, H], f32, kind="ExternalOutput")

    with TileContext(nc) as tc:
        with (
            tc.tile_pool(name="persist", bufs=1) as persist,
        ):
            qT_sb = [persist.tile([P, S], di, tag=f"qT{m}", name=f"qT{m}") for m in range(nMF)]
            kT_sb = [persist.tile([P, S], di, tag=f"kT{m}", name=f"kT{m}") for m in range(nMF)]
            v_sb = [persist.tile([P, VW], di, tag=f"v{st}", name=f"v{st}") for st in range(nST)]
            ctxT_sb = [persist.tile([P, S], di, tag=f"ctx{m}", name=f"ctx{m}") for m in range(nMF)]
            ones_sb = persist.tile([1, P], f32, tag="ones", name="ones")
            nc.vector.memset(ones_sb, 1.0)

            # ---------------- Phase 0: projections ----------------
            with (
                tc.tile_pool(name="p0w", bufs=1) as p0w,
                tc.tile_pool(name="p0x", bufs=1) as p0x,
                tc.tile_pool(name="p0ps", bufs=4, space="PSUM") as p0ps,
            ):
                x_t = [p0x.tile([P, S], di, tag=f"x{kk}", name=f"x{kk}") for kk in range(nKH)]
                for kk in range(nKH):
                    nc.sync.dma_start(x_t[kk], xT[kk * P : (kk + 1) * P, :])
                wq_t = [p0w.tile([P, F], di, tag=f"wq{kk}", name=f"wq{kk}") for kk in range(nKH)]
                wk_t = [p0w.tile([P, F], di, tag=f"wk{kk}", name=f"wk{kk}") for kk in range(nKH)]
                wv_t = [p0w.tile([P, F], di, tag=f"wv{kk}", name=f"wv{kk}") for kk in range(nKH)]
                for kk in range(nKH):
                    nc.sync.dma_start(wq_t[kk], wqT[kk * P : (kk + 1) * P, :])
                    nc.sync.dma_start(wk_t[kk], wkT[kk * P : (kk + 1) * P, :])
                    nc.sync.dma_start(wv_t[kk], wvT[kk * P : (kk + 1) * P, :])

                # qT/kT: [feat part, seq free]
                SC = 512 if S >= 512 else S
                for w_t, dst in ((wq_t, qT_sb), (wk_t, kT_sb)):
                    for mf in range(nMF):
                        for sc in range(S // SC):
                            ps = p0ps.tile([P, SC], f32, tag="p0ps", name="p0ps")
                            for kk in range(nKH):
                                nc.tensor.matmul(
                                    ps,
                                    lhsT=w_t[kk][:, mf * P : (mf + 1) * P],
                                    rhs=x_t[kk][:, sc * SC : (sc + 1) * SC],
                                    start=(kk == 0),
                                    stop=(kk == nKH - 1),
                                )
                            nc.scalar.copy(dst[mf][:, sc * SC : (sc + 1) * SC], ps)
                # v: [seq part, feat free] + ones column per head
                for st in range(nST):
                    ps = p0ps.tile([P, F], f32, tag="p0psv", name="p0psv")
                    for kk in range(nKH):
                        nc.tensor.matmul(
                            ps,
                            lhsT=x_t[kk][:, st * P : (st + 1) * P],
                            rhs=wv_t[kk],
                            start=(kk == 0),
                            stop=(kk == nKH - 1),
                        )
                    for h in range(NH):
                        o = h * (D + 1)
                        nc.vector.tensor_copy(
                            v_sb[st][:, o : o + D], ps[:, h * D : (h + 1) * D]
                        )
                        # ones column (f32r-typed producer; memset can't write f32r)
                        nc.scalar.activation(
                            v_sb[st][:, o + D : o + D + 1],
                            ps[:, 0:1],
                            mybir.ActivationFunctionType.Identity,
                            bias=1.0,
                            scale=0.0,
                        )

            # ---------------- Phase 1: attention ----------------
            with (
                tc.tile_pool(name="p1pt", bufs=2) as p1pt,
                tc.tile_pool(name="p1r", bufs=2) as p1r,
                tc.tile_pool(name="p1sps", bufs=4, space="PSUM") as p1sps,
                tc.tile_pool(name="p1cps", bufs=2, space="PSUM") as p1cps,
                tc.tile_pool(name="p1rps", bufs=1, space="PSUM") as p1rps,
            ):
                for h in range(NH):
                    ft, fo = (h * D) // P, (h * D) % P
                    for qc in range(nQC):
                        qsl = slice(qc * QC, (qc + 1) * QC)
                        pT = p1pt.tile([P, nST, QC], di, tag="pT", name="pT")
                        ctx_ps = p1cps.tile([P, QC], f32, tag="ctxps", name="ctxps")
                        for kk in range(nST):
                            s_ps = p1sps.tile([P, QC], f32, tag="sps", name="sps")
                            nc.tensor.matmul(
                                s_ps,
                                lhsT=kT_sb[ft][fo : fo + D, kk * P : (kk + 1) * P],
                                rhs=qT_sb[ft][fo : fo + D, qsl],
                                start=True,
                                stop=True,
                            )
                            nc.scalar.activation(
                                pT[:, kk, :],
                                s_ps,
                                mybir.ActivationFunctionType.Exp,
                                bias=0.0,
                                scale=float(c_score),
                            )
                            nc.tensor.matmul(
                                ctx_ps[: D + 1, :],
                                lhsT=v_sb[kk][:, h * (D + 1) : (h + 1) * (D + 1)],
                                rhs=pT[:, kk, :],
                                start=(kk == 0),
                                stop=(kk == nST - 1),
                            )
                        rec = p1r.tile([1, QC], f32, tag="rec", name="rec")
                        nc.vector.reciprocal(rec, ctx_ps[D : D + 1, :])
                        r_ps = p1rps.tile([P, QC], f32, tag="rps", name="rps")
                        nc.tensor.matmul(
                            r_ps, lhsT=ones_sb, rhs=rec,
                            start=True, stop=True,
                        )
                        r_sb = p1r.tile([P, QC], f32, tag="rsb", name="rsb")
                        nc.vector.tensor_copy(r_sb, r_ps)
                        for kk in range(nST):
                            nc.vector.tensor_mul(pT[:, kk, :], pT[:, kk, :], r_sb)
                            nc.sync.dma_start(
                                attn_t[h, kk * P : (kk + 1) * P, qsl], pT[:, kk, :]
                            )
                        nc.vector.tensor_mul(
                            ctxT_sb[ft][fo : fo + D, qsl],
                            ctx_ps[:D, :],
                            r_sb[:D, :],
                        )

            # ---------------- Phase 2: output projection ----------------
            with (
                tc.tile_pool(name="p2w", bufs=1) as p2w,
                tc.tile_pool(name="p2o", bufs=3) as p2o,
                tc.tile_pool(name="p2ps", bufs=3, space="PSUM") as p2ps,
            ):
                wo_t = [p2w.tile([P, H], di, tag=f"wo{t}", name=f"wo{t}") for t in range(nMF)]
                for t in range(nMF):
                    nc.sync.dma_start(wo_t[t], woT[t * P : (t + 1) * P, :])
                for st in range(nST):
                    for jc in range(NJ):
                        ps = p2ps.tile([P, JC], f32, tag="p2ps", name="p2ps")
                        for t in range(nMF):
                            nc.tensor.matmul(
                                ps,
                                lhsT=ctxT_sb[t][:, st * P : (st + 1) * P],
                                rhs=wo_t[t][:, jc * JC : (jc + 1) * JC],
                                start=(t == 0),
                                stop=(t == nMF - 1),
                            )
                        osb = p2o.tile([P, JC], f32, tag="osb", name="osb")
                        nc.vector.tensor_scalar_mul(osb, ps, float(c_out))
                        nc.sync.dma_start(
                            out_p[st * P : (st + 1) * P, jc * JC : (jc + 1) * JC],
                            osb,
                        )
    nc.compile()
    return nc


def _host_inputs(hidden_states, Wq, Wk, Wv, Wo, n_cores=8):
    """Per-core input maps + the constants baked into the device program."""
    B, S, H = hidden_states.shape
    heads_per_core = NUM_HEADS * B // n_cores  # 4
    F = heads_per_core * HEAD_DIM  # 256
    sq = np.sign(Wq).astype(np.float32)
    sk = np.sign(Wk).astype(np.float32)
    sv = np.sign(Wv).astype(np.float32)
    so = np.sign(Wo).astype(np.float32)
    a_q = np.abs(Wq).mean(dtype=np.float32)
    a_k = np.abs(Wk).mean(dtype=np.float32)
    a_v = np.abs(Wv).mean(dtype=np.float32)
    a_o = np.abs(Wo).mean(dtype=np.float32)
    c_score = float(a_q) * float(a_k) / float(np.sqrt(HEAD_DIM))
    c_out = float(a_v) * float(a_o)

    xT = [np.ascontiguousarray(hidden_states[b].T) for b in range(B)]
    cores_per_batch = n_cores // B
    in_maps = []
    for c in range(n_cores):
        b, j = divmod(c, cores_per_batch)
        fsl = slice(F * j, F * (j + 1))
        in_maps.append(
            {
                "xT": xT[b],
                "wqT": np.ascontiguousarray(sq[fsl, :].T),
                "wkT": np.ascontiguousarray(sk[fsl, :].T),
                "wvT": np.ascontiguousarray(sv[fsl, :].T),
                "woT": np.ascontiguousarray(so[:, fsl].T),
            }
        )
    return in_maps, c_score, c_out


def _assemble(results, B, S, H, n_cores=8):
    """Gather per-core results into full (out, attn)."""
    heads_per_core = NUM_HEADS * B // n_cores
    cores_per_batch = n_cores // B
    out = np.zeros((B, S, H), dtype=np.float32)
    attn = np.empty((B, NUM_HEADS, S, S), dtype=np.float32)

    def fill(c):
        b, j = divmod(c, cores_per_batch)
        for i in range(heads_per_core):
            h = heads_per_core * j + i
            np.copyto(attn[b, h], results[c]["attn_t"][i].T)

    threads = [threading.Thread(target=fill, args=(c,)) for c in range(n_cores)]
    for t in threads:
        t.start()
    for t in threads:
        t.join()
    for c in range(n_cores):
        b = c // cores_per_batch
        out[b] += results[c]["out_p"]
    return out, attn


def kernel(hidden_states, Wq, Wk, Wv, Wo):
    from concourse.bass_utils import run_bass_kernel_spmd

    B, S, H = hidden_states.shape
    assert (B, S, H) == (2, 2048, 1024), (B, S, H)
    n_cores = 8
    in_maps, c_score, c_out = _host_inputs(
        np.asarray(hidden_states, dtype=np.float32),
        np.asarray(Wq, dtype=np.float32),
        np.asarray(Wk, dtype=np.float32),
        np.asarray(Wv, dtype=np.float32),
        np.asarray(Wo, dtype=np.float32),
        n_cores,
    )
    nc = build_kernel(
        S=S,
        H=H,
        NH=4,
        D=HEAD_DIM,
        QC=512,
        c_score=c_score,
        c_out=c_out,
        use_f32r=os.environ.get("BITNET_F32R", "1") == "1",
    )
    trace = os.environ.get("BITNET_TRACE", "0") == "1"
    res = run_bass_kernel_spmd(
        nc,
        in_maps,
        core_ids=list(range(n_cores)),
        trace=trace,
    )
    if trace:
        kernel.last_results = res
    return _assemble(res.results, B, S, H, n_cores)
